# revision 17
# baseline (speedup 1.0000x reference)
"""Trainium2 Bass kernel for nn_MultiHeadSelfAttention_1_44272522887186.

Math (per (b, c) group, his_length H = 50, words S = 20, d_model D = 400):
  q_a = Q @ Wq.T + bq                      [bch, S, D]   (output #2)
  Qr/Kr/Vr = head_reduce(Q @ W.T + b)      [bch, 20]
  cddr = head_reduce20(cdd)                [bch, 20]
  scoresT = [Kr|cddr] @ [Qr|Kr].T * scale  -> exp -> row-normalize over j
  context = attn @ Vr                      [bch, 20]     (output #1)

Key trick: head_reduce is linear, so Kr/Vr fold into small matrices
M[sd, 20] = sum_j lin_w[s*20+j] * W[20j+t, d] and never require the full
k_a/v_a projections.  The Q-transpose (contraction dim onto partitions) is
done on the TensorEngine in fp32r; the main projection matmul runs fp32r
(1 cycle/row at N=400) and the fold matmul rides the same stationary.

Constraint shaping: fp32r matmuls tolerate only ONE sync-wait (walrus LW
struct), so all PSUM evacuation runs on ScalarE (one semaphore for PE to
track), constants arrive in two packed DMAs absorbed by dummy transposes,
and cross-engine hand-offs always funnel through ACT.

Sharding: data-parallel over batch, 4 of 32 batches per NeuronCore x 8.
"""

import math
import sys

import numpy as np

sys.path.insert(0, "/opt/trn_rl_repo")

import concourse.bass as bass  # noqa: E402
import concourse.mybir as mybir  # noqa: E402
from concourse.bass_utils import run_bass_kernel_spmd  # noqa: E402
from concourse.tile import TileContext  # noqa: E402

# Problem constants (hardcoded; kernel.py must be self-contained).
B, C, H, S, D = 32, 5, 50, 20, 400
NCORES = 8
BPC = B // NCORES          # batches per core = 4
BCH = BPC * C * H          # 1000 rows per core
NT = 20                    # d_k / head-reduce output size
NG = BCH // H              # 20 attention groups per core
TILE = 128
NTILES = (BCH + TILE - 1) // TILE   # 8 (7 full + 104)
DC = 100                   # contraction chunk (4 chunks of 100 = D)
NCHUNK = D // DC           # 4
SCALE = 1.0 / math.sqrt(20.0)

F32 = mybir.dt.float32
F32R = mybir.dt.float32r

# packed fp32r const block columns
WQT_O = 0                    # [0:100, 0:1600]   Wq.T chunk-major
MQKV_O = NCHUNK * D          # [0:100, 1600:6400] folded QKV reduce weights
IDR_O = MQKV_O + S * NCHUNK * 3 * NT   # [*, 6400:6528] identity (f32r)
CBR_W = IDR_O + TILE
# packed fp32 const block columns
ID32_O = 0                   # [*, 0:128] identity (f32)
LIN1_O = TILE                # [*, 128:528] repeated lin1 weights
CB32_W = LIN1_O + D

_CACHE = {}


def _tile_rows(t):
    r0 = t * TILE
    return r0, min(TILE, BCH - r0)


def build_nc():
    nc = bass.Bass()

    q_in = nc.declare_dram_parameter("q", [BCH, S * D], F32R, isOutput=False)
    cdd_in = nc.declare_dram_parameter("cdd", [BCH, D], F32, isOutput=False)
    cbr_in = nc.declare_dram_parameter("cbr", [TILE, CBR_W], F32R, isOutput=False)
    cb32_in = nc.declare_dram_parameter("cb32", [TILE, CB32_W], F32, isOutput=False)

    qa_out = nc.declare_dram_parameter("qa", [BCH, S, D], F32, isOutput=True)
    ctx_out = nc.declare_dram_parameter("ctx", [BCH, NT], F32, isOutput=True)

    ExpF = mybir.ActivationFunctionType.Exp
    CopyF = mybir.ActivationFunctionType.Copy
    Mult = mybir.AluOpType.mult

    with TileContext(nc) as tc:
        with (
            tc.tile_pool(name="consts", bufs=1) as consts,
            tc.tile_pool(name="slab", bufs=2) as slabp,
            tc.tile_pool(name="qt", bufs=3) as qtp,
            tc.tile_pool(name="qa", bufs=3) as qap,
            tc.tile_pool(name="small", bufs=3) as smallp,
            tc.tile_pool(name="tpsum", bufs=2, space="PSUM") as tpsum,
            tc.tile_pool(name="mpsum", bufs=3, space="PSUM") as mpsum,
            tc.tile_pool(name="fpsum", bufs=1, space="PSUM") as fpsum,
            tc.tile_pool(name="apsum", bufs=2, space="PSUM") as apsum,
        ):
            # ---- persistent SBUF tensors ----
            cbr_sb = consts.tile([TILE, CBR_W], F32R)
            cb32_sb = consts.tile([TILE, CB32_W], F32)
            # per-tile layout: cols [0:20 Qr, 20:40 Kr, 40:60 cddr, 60:80 Vr]
            qkvr_sb = consts.tile([TILE, NTILES * 80], F32)
            at_sb = consts.tile([40, NTILES * TILE], F32)
            bt_sb = consts.tile([40, NTILES * TILE], F32)
            vrt_sb = consts.tile([NT, NTILES * TILE], F32)

            nc.sync.dma_start(out=cbr_sb[:, :], in_=cbr_in[:, :])
            nc.sync.dma_start(out=cb32_sb[:, :], in_=cb32_in[:, :])

            wqt = cbr_sb[0:DC, WQT_O : WQT_O + NCHUNK * D]
            mqkv = cbr_sb[0:DC, MQKV_O : MQKV_O + S * NCHUNK * 3 * NT]
            ident = cbr_sb[:, IDR_O : IDR_O + TILE]
            ident32 = cb32_sb[:, ID32_O : ID32_O + TILE]
            lin1 = cb32_sb[:, LIN1_O : LIN1_O + D]

            # dummy absorbers: let PE/DVE observe the const DMA semaphores
            # once, so later matmuls never need a second wait slot.
            d_ps = tpsum.tile([32, 32], F32R, tag="tps")
            nc.tensor.transpose(
                d_ps[:, :], ident[0:32, 0:32], ident[0:32, 0:32]
            )
            d32_ps = apsum.tile([32, 32], F32, tag="aps")
            nc.tensor.transpose(
                d32_ps[:, :], ident32[0:32, 0:32], ident32[0:32, 0:32]
            )
            dve_sb = smallp.tile([1, 4], F32, tag="dummy")
            nc.vector.tensor_copy(out=dve_sb[0:1, 0:1], in_=lin1[0:1, 0:1])

            # ---- main loop: projection + folded head-reduction ----
            for t in range(NTILES):
                r0, p = _tile_rows(t)
                q_sb = slabp.tile([TILE, S * D], F32R, tag="slab")
                nc.sync.dma_start(out=q_sb[0:p, :], in_=q_in[r0 : r0 + p, :])

                fold_ps = fpsum.tile([TILE, 3 * NT], F32, tag="fold")

                for s in range(S):
                    # transpose 4 chunks of Q[:, s, :] -> [DC, p] each
                    t_ps = tpsum.tile([DC, 512], F32R, tag="tps")
                    for c in range(NCHUNK):
                        nc.tensor.transpose(
                            t_ps[:, c * TILE : c * TILE + p],
                            q_sb[0:p, s * D + c * DC : s * D + (c + 1) * DC],
                            ident[0:p, 0:p],
                        )
                    qt_sb = qtp.tile([DC, 512], F32R, tag="qt")
                    nc.scalar.activation(qt_sb[:, :], t_ps[:, :], CopyF)

                    main_ps = mpsum.tile([TILE, D], F32, tag="main")
                    for c in range(NCHUNK):
                        lhsT = qt_sb[:, c * TILE : c * TILE + p]
                        nc.tensor.matmul(
                            main_ps[0:p, :],
                            lhsT,
                            wqt[:, c * D : (c + 1) * D],
                            start=(c == 0),
                            stop=(c == NCHUNK - 1),
                        )
                        nc.tensor.matmul(
                            fold_ps[0:p, :],
                            lhsT,
                            mqkv[
                                :,
                                (s * NCHUNK + c) * 3 * NT : (s * NCHUNK + c + 1)
                                * 3
                                * NT,
                            ],
                            start=(s == 0 and c == 0),
                            stop=(s == S - 1 and c == NCHUNK - 1),
                            skip_group_check=True,
                        )

                    qa_sb = qap.tile([TILE, D], F32, tag="qa")
                    nc.scalar.activation(qa_sb[0:p, :], main_ps[0:p, :], CopyF)
                    nc.sync.dma_start(
                        out=qa_out[r0 : r0 + p, s, :], in_=qa_sb[0:p, :]
                    )

                # fold psum -> qkvr (Qr,Kr cols 0:40 / Vr cols 60:80)
                nc.scalar.activation(
                    qkvr_sb[0:p, t * 80 : t * 80 + 40], fold_ps[0:p, 0:40], CopyF
                )
                nc.scalar.activation(
                    qkvr_sb[0:p, t * 80 + 60 : t * 80 + 80],
                    fold_ps[0:p, 40:60],
                    CopyF,
                )

                # cddr for this tile: elementwise * lin1, strided reduce (DVE),
                # then an ACT hop into qkvr so PE only ever waits on ACT.
                cdd_sb = smallp.tile([TILE, D], F32, tag="cdd")
                nc.sync.dma_start(out=cdd_sb[0:p, :], in_=cdd_in[r0 : r0 + p, :])
                cw_sb = smallp.tile([TILE, D], F32, tag="cw")
                nc.vector.tensor_tensor(
                    out=cw_sb[0:p, :],
                    in0=cdd_sb[0:p, :],
                    in1=lin1[0:p, :],
                    op=Mult,
                )
                cr_sb = smallp.tile([TILE, NT], F32, tag="cr")
                cw_v = cw_sb[0:p, :].rearrange("p (n t) -> p t n", n=NT, t=NT)
                nc.vector.tensor_reduce(
                    out=cr_sb[0:p, :],
                    in_=cw_v,
                    axis=mybir.AxisListType.X,
                    op=mybir.AluOpType.add,
                )
                nc.scalar.activation(
                    qkvr_sb[0:p, t * 80 + 40 : t * 80 + 60], cr_sb[0:p, :], CopyF
                )

                # transposed views for attention: AT = [Qr;Kr], BT = [Kr;cddr]
                a_ps = apsum.tile([40, TILE], F32, tag="aps")
                nc.tensor.transpose(
                    a_ps[:, 0:p],
                    qkvr_sb[0:p, t * 80 : t * 80 + 40],
                    ident32[0:p, 0:p],
                )
                nc.scalar.activation(
                    at_sb[:, t * TILE : t * TILE + p], a_ps[:, 0:p], CopyF
                )
                b_ps = apsum.tile([40, TILE], F32, tag="aps")
                nc.tensor.transpose(
                    b_ps[:, 0:p],
                    qkvr_sb[0:p, t * 80 + 20 : t * 80 + 60],
                    ident32[0:p, 0:p],
                )
                nc.scalar.activation(
                    bt_sb[:, t * TILE : t * TILE + p], b_ps[:, 0:p], CopyF
                )
                v_ps = apsum.tile([40, TILE], F32, tag="aps")
                nc.tensor.transpose(
                    v_ps[0:NT, 0:p],
                    qkvr_sb[0:p, t * 80 + 60 : t * 80 + 80],
                    ident32[0:p, 0:p],
                )
                nc.scalar.activation(
                    vrt_sb[:, t * TILE : t * TILE + p], v_ps[0:NT, 0:p], CopyF
                )

            # ---- attention tail: 20 groups of 50 ----
            for g in range(NG):
                g0 = g * H
                # scoresT [j, i] for the context matmul
                st_ps = apsum.tile([H, H], F32, tag="aps")
                nc.tensor.matmul(
                    st_ps[:, :],
                    bt_sb[:, g0 : g0 + H],
                    at_sb[:, g0 : g0 + H],
                    start=True,
                    stop=True,
                )
                sc_sb = smallp.tile([H, H], F32, tag="sc")
                nc.scalar.activation(sc_sb[:, :], st_ps[:, :], ExpF, scale=SCALE)

                # scores [i, j] only to get row sums via ACT accumulate
                s_ps = apsum.tile([H, H], F32, tag="aps")
                nc.tensor.matmul(
                    s_ps[:, :],
                    at_sb[:, g0 : g0 + H],
                    bt_sb[:, g0 : g0 + H],
                    start=True,
                    stop=True,
                )
                junk_sb = smallp.tile([H, H], F32, tag="junk")
                rs_sb = smallp.tile([H, 3], F32, tag="rs")
                nc.scalar.activation(
                    junk_sb[:, :],
                    s_ps[:, :],
                    ExpF,
                    scale=SCALE,
                    accum_out=rs_sb[:, 0:1],
                )

                # Vr for this group back to [j, t'] layout
                v_ps = apsum.tile([H, NT], F32, tag="aps")
                nc.tensor.transpose(
                    v_ps[:, :],
                    vrt_sb[:, g0 : g0 + H],
                    ident32[0:NT, 0:NT],
                )
                vo_sb = smallp.tile([H, NT], F32, tag="vo")
                nc.scalar.activation(vo_sb[:, :], v_ps[:, :], CopyF)

                cx_ps = apsum.tile([H, NT], F32, tag="aps")
                nc.tensor.matmul(
                    cx_ps[:, :], sc_sb[:, :], vo_sb[:, :], start=True, stop=True
                )
                nc.vector.tensor_scalar_add(
                    out=rs_sb[:, 1:2], in0=rs_sb[:, 0:1], scalar1=1e-8
                )
                nc.vector.reciprocal(out=rs_sb[:, 2:3], in_=rs_sb[:, 1:2])
                cx_sb = smallp.tile([H, NT], F32, tag="cx")
                nc.scalar.activation(
                    cx_sb[:, :], cx_ps[:, :], CopyF, scale=rs_sb[:, 2:3]
                )
                nc.sync.dma_start(out=ctx_out[g0 : g0 + H, :], in_=cx_sb[:, :])

    _legalize_waits(nc)
    return nc


# walrus encodes exactly ONE sem-wait per hardware instruction.  Tile can
# emit more: (a) same-engine waits, redundant for pc-ordered engines
# (PE/ACT/DVE completions are in program order), which we drop; (b) genuine
# multi-sem waits, which we peel onto NoOp instructions inserted just
# before on the same engine queue (equivalent: nothing runs in between).
_ENG_SELF_PREFIX = {
    "PE": "PE_",
    "Activation": "Activation_",
    "DVE": "DVE_",
    "Pool": "Pool_",
    "SP": "SP_",
}


def _legalize_waits(nc):
    noop_id = [0]
    for f in nc.m.functions:
        for blk in f.blocks:
            out = []
            for inst in blk.instructions:
                si = getattr(inst, "sync_info", None)
                if si is not None and si.on_wait and len(si.on_wait) > 1:
                    eng = str(inst.engine).split(".")[-1]
                    pref = _ENG_SELF_PREFIX.get(eng)
                    waits = list(si.on_wait)
                    kept = [
                        w
                        for w in waits
                        if not (
                            pref
                            and w.ant_name
                            and w.ant_name.startswith(pref)
                            and w.wait_reg is None
                        )
                    ]
                    if not kept:
                        kept = waits[-1:]
                    for w in kept[:-1]:
                        noop_id[0] += 1
                        noop = mybir.InstNoOp(
                            name=f"I-waitfix-{noop_id[0]}",
                            engine=inst.engine,
                            sync_info=mybir.SyncInfo(on_wait=[w], on_update=[]),
                            ins=[],
                            outs=[],
                        )
                        out.append(noop)
                    inst.sync_info = mybir.SyncInfo(
                        on_wait=kept[-1:], on_update=list(si.on_update or [])
                    )
                out.append(inst)
            blk.instructions[:] = out


def _host_consts(Wq, Wk, Wv, lin_w, lin1_w):
    lw2 = np.asarray(lin_w, np.float64).reshape(S, NT)  # A[s, j]
    ms = []
    for W in (Wq, Wk, Wv):
        W3 = np.asarray(W, np.float64).reshape(NT, NT, D)  # [j, t, d]
        m = np.einsum("sj,jtd->sdt", lw2, W3)  # [s, d, t]
        ms.append(m.reshape(S * D, NT))
    mqkv = np.concatenate(ms, axis=1)  # [8000, 60]
    mqkv_l = (
        mqkv.reshape(S, NCHUNK, DC, 3 * NT)
        .transpose(2, 0, 1, 3)
        .reshape(DC, S * NCHUNK * 3 * NT)
        .astype(np.float32)
    )
    wqt_l = (
        np.asarray(Wq, np.float32)
        .T.reshape(NCHUNK, DC, D)
        .transpose(1, 0, 2)
        .reshape(DC, NCHUNK * D)
    )
    ident = np.eye(TILE, dtype=np.float32)

    cbr = np.zeros((TILE, CBR_W), np.float32)
    cbr[0:DC, WQT_O : WQT_O + NCHUNK * D] = wqt_l
    cbr[0:DC, MQKV_O : MQKV_O + S * NCHUNK * 3 * NT] = mqkv_l
    cbr[:, IDR_O : IDR_O + TILE] = ident

    cb32 = np.zeros((TILE, CB32_W), np.float32)
    cb32[:, ID32_O : ID32_O + TILE] = ident
    cb32[:, LIN1_O : LIN1_O + D] = np.repeat(
        np.asarray(lin1_w, np.float32), NT
    )[None, :]
    return np.ascontiguousarray(cbr), np.ascontiguousarray(cb32)


def _get_nc():
    if "nc" not in _CACHE:
        _CACHE["nc"] = build_nc()
    return _CACHE["nc"]


def run(inputs, trace=False, **kw):
    Q = np.ascontiguousarray(np.asarray(inputs["Q"], np.float32))
    cdd = np.ascontiguousarray(np.asarray(inputs["cdd"], np.float32))
    for name in ("bq", "bk", "bv", "lin_b", "lin1_b"):
        if not np.all(np.asarray(inputs[name]) == 0):
            raise NotImplementedError(f"nonzero bias {name} not supported")

    cbr, cb32 = _host_consts(
        inputs["Wq"], inputs["Wk"], inputs["Wv"], inputs["lin_w"], inputs["lin1_w"]
    )

    in_maps = []
    for i in range(NCORES):
        qs = Q[i * BPC : (i + 1) * BPC].reshape(BCH, S * D)
        cs = cdd[i * BPC : (i + 1) * BPC].reshape(BCH, D)
        in_maps.append(
            {
                "q": np.ascontiguousarray(qs),
                "cdd": np.ascontiguousarray(cs),
                "cbr": cbr,
                "cb32": cb32,
            }
        )

    res = run_bass_kernel_spmd(
        _get_nc(), in_maps, list(range(NCORES)), trace=trace, **kw
    )
    qa = np.stack([np.asarray(r["qa"]) for r in res.results])
    ctx = np.stack([np.asarray(r["ctx"]) for r in res.results])
    q_a = qa.reshape(B, C, H, S, D)
    context = ctx.reshape(B, C, H, NT)
    return (context, q_a), res


def kernel(**inputs):
    (context, q_a), _ = run(inputs, trace=False)
    return context, q_a


# revision 35
# speedup vs baseline: 192.7842x; 192.7842x over previous
"""Trainium2 Bass kernel for nn_MultiHeadSelfAttention_1_44272522887186.

Math (per (b, c) group, his_length H = 50, words S = 20, d_model D = 400):
  q_a = Q @ Wq.T + bq                      [bch, S, D]   (output #2)
  Qr/Kr/Vr = head_reduce(Q @ W.T + b)      [bch, 20]
  cddr = head_reduce20(cdd)                [bch, 20]
  scoresT = [Kr|cddr] @ [Qr|Kr].T * scale  -> exp -> row-normalize over j
  context = attn @ Vr                      [bch, 20]     (output #1)

Key trick: head_reduce is linear, so Kr/Vr fold into small matrices
M[sd, 20] = sum_j lin_w[s*20+j] * W[20j+t, d] and never require the full
k_a/v_a projections.  The Q-transpose (contraction dim onto partitions) is
done on the TensorEngine in fp32r; the main projection matmul runs fp32r
(1 cycle/row at N=400) and the fold matmul rides the same stationary.

Constraint shaping: fp32r matmuls tolerate only ONE sync-wait (walrus LW
struct), so all PSUM evacuation runs on ScalarE (one semaphore for PE to
track), constants arrive in two packed DMAs absorbed by dummy transposes,
and cross-engine hand-offs always funnel through ACT.

Sharding: data-parallel over batch, 4 of 32 batches per NeuronCore x 8.
"""

import math
import sys

import numpy as np

sys.path.insert(0, "/opt/trn_rl_repo")

import concourse.bass as bass  # noqa: E402
import concourse.mybir as mybir  # noqa: E402
from concourse.bass_utils import run_bass_kernel_spmd  # noqa: E402
from concourse.tile import TileContext  # noqa: E402

# Problem constants (hardcoded; kernel.py must be self-contained).
B, C, H, S, D = 32, 5, 50, 20, 400
NCORES = 8
BPC = B // NCORES          # batches per core = 4
BCH = BPC * C * H          # 1000 rows per core
NT = 20                    # d_k / head-reduce output size
NG = BCH // H              # 20 attention groups per core
TILE = 128
NTILES = (BCH + TILE - 1) // TILE   # 8 (7 full + 104)
DC = 100                   # contraction chunk (4 chunks of 100 = D)
NCHUNK = D // DC           # 4
SCALE = 1.0 / math.sqrt(20.0)

F32 = mybir.dt.float32
F32R = mybir.dt.float32r

# packed fp32r const block columns
WQT_O = 0                    # [0:100, 0:1600]   Wq.T chunk-major
MQKV_O = NCHUNK * D          # [0:100, +6720] folded QKV weights, 84-col blocks
                             # (rows Qr@0:20 Kr@32:52 Vr@64:84, 32-aligned)
IDR_O = MQKV_O + S * NCHUNK * 84       # [*, +128] identity (f32r)
CBR_W = IDR_O + TILE
# packed fp32 const block columns
ID32_O = 0                   # [*, 0:128] identity (f32)
LIN1_O = TILE                # [*, 128:528] repeated lin1 weights
ONES_O = LIN1_O + D          # [*, 528:529] ones column
CB32_W = ONES_O + 1

_CACHE = {}


def build_nc(legalize=True):
    nc = bass.Bass()

    q_in = nc.declare_dram_parameter("q", [NTILES * TILE, S * D], F32R, isOutput=False)
    # cdd padded to NTILES*TILE rows so strided pair DMAs stay in bounds
    cdd_in = nc.declare_dram_parameter("cdd", [NTILES * TILE, D], F32, isOutput=False)
    cbr_in = nc.declare_dram_parameter("cbr", [TILE, CBR_W], F32R, isOutput=False)
    cb32_in = nc.declare_dram_parameter("cb32", [TILE, CB32_W], F32, isOutput=False)

    qa_out = nc.declare_dram_parameter("qa", [NTILES * TILE, S, D], F32, isOutput=True)
    ctx_out = nc.declare_dram_parameter("ctx", [BCH, NT], F32, isOutput=True)

    ExpF = mybir.ActivationFunctionType.Exp
    CopyF = mybir.ActivationFunctionType.Copy
    Mult = mybir.AluOpType.mult
    SH = S // 2  # s-half size (10)

    with TileContext(nc) as tc:
        with (
            tc.tile_pool(name="consts", bufs=1) as consts,
            tc.tile_pool(name="slab", bufs=4) as slabp,
            tc.tile_pool(name="qt", bufs=3) as qtp,
            tc.tile_pool(name="qa", bufs=3) as qap,
            tc.tile_pool(name="small", bufs=4) as smallp,
            tc.tile_pool(name="tpsum", bufs=2, space="PSUM") as tpsum,
            tc.tile_pool(name="mpsum", bufs=2, space="PSUM") as mpsum,
            tc.tile_pool(name="fpsum", bufs=2, space="PSUM") as fpsum,
            tc.tile_pool(name="apsum", bufs=2, space="PSUM") as apsum,
        ):
            # ---- persistent SBUF tensors ----
            cbr_sb = consts.tile([TILE, CBR_W], F32R)
            cb32_sb = consts.tile([TILE, CB32_W], F32)
            # head-reduced tensors, transposed: [20 t', bch] blocks
            # cols: Qr @ QKT_QR, Kr @ QKT_KR, Vr @ QKT_VR, cddr @ QKT_CD
            qkt_sb = consts.tile([NT, 4 * NTILES * TILE], F32)
            ctx_all = consts.tile([H, NG * NT], F32)

            nc.sync.dma_start(
                out=cbr_sb[:, 0 : WQT_O + NCHUNK * D],
                in_=cbr_in[:, 0 : WQT_O + NCHUNK * D],
            )
            nc.sync.dma_start(
                out=cbr_sb[:, IDR_O : IDR_O + TILE],
                in_=cbr_in[:, IDR_O : IDR_O + TILE],
            )
            nc.sync.dma_start(out=cb32_sb[:, :], in_=cb32_in[:, :])
            nc.sync.dma_start(
                out=cbr_sb[:, MQKV_O : MQKV_O + S * NCHUNK * 84],
                in_=cbr_in[:, MQKV_O : MQKV_O + S * NCHUNK * 84],
            )

            wqt = cbr_sb[0:DC, WQT_O : WQT_O + NCHUNK * D]
            mqkv = cbr_sb[0:DC, MQKV_O : MQKV_O + S * NCHUNK * 84]
            ident = cbr_sb[:, IDR_O : IDR_O + TILE]
            ident32 = cb32_sb[:, ID32_O : ID32_O + TILE]
            lin1 = cb32_sb[:, LIN1_O : LIN1_O + D]

            NB = NTILES * TILE  # 1024 padded bch
            QR_O, KR_O, VR_O, CD_O = 0, NB, 2 * NB, 3 * NB

            # dummy absorbers: let PE/DVE observe the const DMA semaphores
            # once, so later matmuls never need a second wait slot.
            d_ps = tpsum.tile([32, 32], F32R, tag="tps")
            nc.tensor.transpose(
                d_ps[:, :], ident[0:32, 0:32], ident[0:32, 0:32]
            )
            d32_ps = apsum.tile([32, 32], F32, tag="aps")
            nc.tensor.transpose(
                d32_ps[:, :], ident32[0:32, 0:32], ident32[0:32, 0:32]
            )
            dve_sb = smallp.tile([1, 4], F32, tag="dummy")
            nc.vector.tensor_copy(out=dve_sb[0:1, 0:1], in_=lin1[0:1, 0:1])

            # ---- main loop over tile pairs ----
            for tp in range(NTILES // 2):
                ta, tb = 2 * tp, 2 * tp + 1
                r0a, pa = ta * TILE, TILE
                r0b, pb = tb * TILE, TILE

                # fold accumulator [84, 256]: rows Qr@0:20 Kr@32:52 Vr@64:84,
                # cols = bch r0a..r0a+256 (tail cols garbage for pb<128)
                fold_ps = fpsum.tile([84, 2 * TILE], F32, tag="fold")

                for half in range(2):
                    sl_a = slabp.tile([TILE, SH * D], F32R, tag="slab")
                    nc.sync.dma_start(
                        out=sl_a[0:pa, :],
                        in_=q_in[r0a : r0a + pa, half * SH * D : (half + 1) * SH * D],
                    )
                    sl_b = slabp.tile([TILE, SH * D], F32R, tag="slab")
                    nc.sync.dma_start(
                        out=sl_b[0:pb, :],
                        in_=q_in[r0b : r0b + pb, half * SH * D : (half + 1) * SH * D],
                    )
                    qa_st_a = qap.tile([TILE, SH * D], F32, tag="qa")
                    qa_st_b = qap.tile([TILE, SH * D], F32, tag="qa")

                    for sh in range(SH):
                        s = half * SH + sh
                        # transpose 4 chunks of each tile -> psum [100, 4*128]
                        tps_a = tpsum.tile([DC, 512], F32R, tag="tps")
                        for c in range(NCHUNK):
                            nc.tensor.transpose(
                                tps_a[:, c * TILE : c * TILE + pa],
                                sl_a[0:pa, sh * D + c * DC : sh * D + (c + 1) * DC],
                                ident[0:pa, 0:pa],
                            )
                        tps_b = tpsum.tile([DC, 512], F32R, tag="tps")
                        for c in range(NCHUNK):
                            nc.tensor.transpose(
                                tps_b[:, c * TILE : c * TILE + pb],
                                sl_b[0:pb, sh * D + c * DC : sh * D + (c + 1) * DC],
                                ident[0:pb, 0:pb],
                            )
                        # pack both tiles: block (c, i) at col (2c+i)*128
                        qt_sb = qtp.tile([DC, 1024], F32R, tag="qt")
                        nc.scalar.activation(
                            qt_sb[:, :].rearrange(
                                "p (c i n) -> p (c i) n", c=NCHUNK, i=2, n=TILE
                            )[:, 0::2, :],
                            tps_a[:, :].rearrange(
                                "p (c n) -> p c n", c=NCHUNK, n=TILE
                            ),
                            CopyF,
                        )
                        nc.scalar.activation(
                            qt_sb[:, :].rearrange(
                                "p (c i n) -> p (c i) n", c=NCHUNK, i=2, n=TILE
                            )[:, 1::2, :],
                            tps_b[:, :].rearrange(
                                "p (c n) -> p c n", c=NCHUNK, n=TILE
                            ),
                            CopyF,
                        )

                        mps_a = mpsum.tile([TILE, D], F32, tag="main")
                        mps_b = mpsum.tile([TILE, D], F32, tag="main")
                        for c in range(NCHUNK):
                            nc.tensor.matmul(
                                mps_a[0:pa, :],
                                qt_sb[:, (2 * c) * TILE : (2 * c) * TILE + pa],
                                wqt[:, c * D : (c + 1) * D],
                                start=(c == 0),
                                stop=(c == NCHUNK - 1),
                            )
                            nc.tensor.matmul(
                                mps_b[0:pb, :],
                                qt_sb[:, (2 * c + 1) * TILE : (2 * c + 1) * TILE + pb],
                                wqt[:, c * D : (c + 1) * D],
                                start=(c == 0),
                                stop=(c == NCHUNK - 1),
                                skip_group_check=True,
                            )
                            nc.tensor.matmul(
                                fold_ps[:, :],
                                mqkv[:, (s * NCHUNK + c) * 84 : (s * NCHUNK + c + 1) * 84],
                                qt_sb[:, (2 * c) * TILE : (2 * c + 2) * TILE],
                                start=(s == 0 and c == 0),
                                stop=(s == S - 1 and c == NCHUNK - 1),
                                skip_group_check=True,
                            )

                        nc.vector.tensor_copy(
                            out=qa_st_a[0:pa, sh * D : (sh + 1) * D],
                            in_=mps_a[0:pa, :],
                        )
                        nc.vector.tensor_copy(
                            out=qa_st_b[0:pb, sh * D : (sh + 1) * D],
                            in_=mps_b[0:pb, :],
                        )

                    s0 = half * SH
                    nc.sync.dma_start(
                        out=qa_out[r0a : r0a + pa, s0 : s0 + SH, :],
                        in_=qa_st_a[0:pa, :].rearrange("p (s d) -> p s d", s=SH, d=D),
                    )
                    nc.sync.dma_start(
                        out=qa_out[r0b : r0b + pb, s0 : s0 + SH, :],
                        in_=qa_st_b[0:pb, :].rearrange("p (s d) -> p s d", s=SH, d=D),
                    )

                # evacuate fold -> transposed blocks (32-aligned shifts)
                b0 = 2 * tp * TILE
                nc.scalar.activation(
                    qkt_sb[:, QR_O + b0 : QR_O + b0 + 2 * TILE],
                    fold_ps[0:NT, :],
                    CopyF,
                )
                nc.scalar.activation(
                    qkt_sb[:, KR_O + b0 : KR_O + b0 + 2 * TILE],
                    fold_ps[32 : 32 + NT, :],
                    CopyF,
                )
                nc.scalar.activation(
                    qkt_sb[:, VR_O + b0 : VR_O + b0 + 2 * TILE],
                    fold_ps[64 : 64 + NT, :],
                    CopyF,
                )

                # cddr for both tiles: mult+reduce on DVE, transpose on PE,
                # ACT copy into the cddr block.
                cdd_sb = smallp.tile([TILE, 2 * D], F32, tag="cdd")
                nc.sync.dma_start(
                    out=cdd_sb[:, :].rearrange("p (t d) -> p t d", t=2, d=D),
                    in_=cdd_in[b0 : b0 + 2 * TILE, :].rearrange(
                        "(t p) d -> p t d", t=2, p=TILE
                    ),
                )
                for i, (t, p) in enumerate(((ta, pa), (tb, pb))):
                    cw_sb = smallp.tile([TILE, D], F32, tag="cw")
                    nc.vector.tensor_tensor(
                        out=cw_sb[0:p, :],
                        in0=cdd_sb[0:p, i * D : (i + 1) * D],
                        in1=lin1[0:p, :],
                        op=Mult,
                    )
                    cr_sb = smallp.tile([TILE, NT], F32, tag="cr")
                    cw_v = cw_sb[0:p, :].rearrange("p (n t) -> p t n", n=NT, t=NT)
                    nc.vector.tensor_reduce(
                        out=cr_sb[0:p, :],
                        in_=cw_v,
                        axis=mybir.AxisListType.X,
                        op=mybir.AluOpType.add,
                    )
                    c_ps = apsum.tile([NT, TILE], F32, tag="aps")
                    nc.tensor.transpose(
                        c_ps[:, 0:p], cr_sb[0:p, :], ident32[0:p, 0:p]
                    )
                    nc.scalar.activation(
                        qkt_sb[:, CD_O + t * TILE : CD_O + t * TILE + p],
                        c_ps[:, 0:p],
                        CopyF,
                    )

            # ---- attention tail: 20 groups of 50 ----
            for g in range(NG):
                g0 = g * H
                qr_g = qkt_sb[:, QR_O + g0 : QR_O + g0 + H]
                kr_g = qkt_sb[:, KR_O + g0 : KR_O + g0 + H]
                vr_g = qkt_sb[:, VR_O + g0 : VR_O + g0 + H]
                cd_g = qkt_sb[:, CD_O + g0 : CD_O + g0 + H]

                # scoresT [j, i] = Kr[j].Qr[i] + cddr[j].Kr[i]
                st_ps = apsum.tile([H, H], F32, tag="aps")
                nc.tensor.matmul(
                    st_ps[:, :], kr_g, qr_g, start=True, stop=False,
                    skip_group_check=True,
                )
                nc.tensor.matmul(
                    st_ps[:, :], cd_g, kr_g, start=False, stop=True,
                    skip_group_check=True,
                )
                sc_sb = smallp.tile([H, H], F32, tag="sc")
                nc.scalar.activation(sc_sb[:, :], st_ps[:, :], ExpF, scale=SCALE)

                # Vr for this group back to [j, t'] layout; col 20 = ones so
                # the context matmul also produces the exp row sums
                v_ps = apsum.tile([H, NT], F32, tag="aps")
                nc.tensor.transpose(v_ps[:, :], vr_g, ident32[0:NT, 0:NT])
                vo_sb = smallp.tile([H, NT + 1], F32, tag="vo")
                nc.scalar.activation(vo_sb[:, 0:NT], v_ps[:, :], CopyF)
                nc.scalar.activation(
                    vo_sb[:, NT : NT + 1], cb32_sb[0:H, ONES_O : ONES_O + 1], CopyF
                )

                cx_ps = apsum.tile([H, NT + 1], F32, tag="aps")
                nc.tensor.matmul(
                    cx_ps[:, :], sc_sb[:, :], vo_sb[:, :], start=True, stop=True
                )
                rs_sb = smallp.tile([H, 3], F32, tag="rs")
                nc.vector.tensor_scalar_add(
                    out=rs_sb[:, 1:2], in0=cx_ps[:, NT : NT + 1], scalar1=1e-8
                )
                nc.vector.reciprocal(out=rs_sb[:, 2:3], in_=rs_sb[:, 1:2])
                nc.scalar.activation(
                    ctx_all[:, g * NT : (g + 1) * NT],
                    cx_ps[:, 0:NT],
                    CopyF,
                    scale=rs_sb[:, 2:3],
                )

            nc.sync.dma_start(
                out=ctx_out[:, :].rearrange("(g h) t -> h g t", g=NG, h=H),
                in_=ctx_all[:, :].rearrange("h (g t) -> h g t", g=NG, t=NT),
            )

    if legalize:
        _legalize_waits(nc)
    return nc


# walrus encodes exactly ONE sem-wait per hardware instruction.  Tile can
# emit more: (a) same-engine waits, redundant for pc-ordered engines
# (PE/ACT/DVE completions are in program order), which we drop; (b) genuine
# multi-sem waits, which we peel onto NoOp instructions inserted just
# before on the same engine queue (equivalent: nothing runs in between).
_ENG_SELF_PREFIX = {
    "PE": "PE_",
    "Activation": "Activation_",
    "DVE": "DVE_",
    "Pool": "Pool_",
    "SP": "SP_",
}


def _legalize_waits(nc):
    noop_id = [0]
    for f in nc.m.functions:
        for blk in f.blocks:
            out = []
            for inst in blk.instructions:
                si = getattr(inst, "sync_info", None)
                if si is not None and si.on_wait and len(si.on_wait) > 1:
                    eng = str(inst.engine).split(".")[-1]
                    pref = _ENG_SELF_PREFIX.get(eng)
                    waits = list(si.on_wait)
                    kept = [
                        w
                        for w in waits
                        if not (
                            pref
                            and w.ant_name
                            and w.ant_name.startswith(pref)
                            and w.wait_reg is None
                        )
                    ]
                    if not kept:
                        kept = waits[-1:]
                    for w in kept[:-1]:
                        noop_id[0] += 1
                        noop = mybir.InstNoOp(
                            name=f"I-waitfix-{noop_id[0]}",
                            engine=inst.engine,
                            sync_info=mybir.SyncInfo(on_wait=[w], on_update=[]),
                            ins=[],
                            outs=[],
                        )
                        out.append(noop)
                    inst.sync_info = mybir.SyncInfo(
                        on_wait=kept[-1:], on_update=list(si.on_update or [])
                    )
                out.append(inst)
            blk.instructions[:] = out


def _host_consts(Wq, Wk, Wv, lin_w, lin1_w):
    lw2 = np.asarray(lin_w, np.float64).reshape(S, NT)  # A[s, j]
    mqkv = np.zeros((S * D, 84), np.float64)
    for k, W in enumerate((Wq, Wk, Wv)):
        W3 = np.asarray(W, np.float64).reshape(NT, NT, D)  # [j, t, d]
        m = np.einsum("sj,jtd->sdt", lw2, W3)  # [s, d, t]
        mqkv[:, 32 * k : 32 * k + NT] = m.reshape(S * D, NT)
    mqkv_l = (
        mqkv.reshape(S, NCHUNK, DC, 84)
        .transpose(2, 0, 1, 3)
        .reshape(DC, S * NCHUNK * 84)
        .astype(np.float32)
    )
    wqt_l = (
        np.asarray(Wq, np.float32)
        .T.reshape(NCHUNK, DC, D)
        .transpose(1, 0, 2)
        .reshape(DC, NCHUNK * D)
    )
    ident = np.eye(TILE, dtype=np.float32)

    cbr = np.zeros((TILE, CBR_W), np.float32)
    cbr[0:DC, WQT_O : WQT_O + NCHUNK * D] = wqt_l
    cbr[0:DC, MQKV_O : MQKV_O + S * NCHUNK * 84] = mqkv_l
    cbr[:, IDR_O : IDR_O + TILE] = ident

    cb32 = np.zeros((TILE, CB32_W), np.float32)
    cb32[:, ID32_O : ID32_O + TILE] = ident
    cb32[:, LIN1_O : LIN1_O + D] = np.repeat(
        np.asarray(lin1_w, np.float32), NT
    )[None, :]
    cb32[:, ONES_O] = 1.0
    return np.ascontiguousarray(cbr), np.ascontiguousarray(cb32)


def _get_nc():
    if "nc" not in _CACHE:
        _CACHE["nc"] = build_nc()
    return _CACHE["nc"]


def run(inputs, trace=False, **kw):
    Q = np.ascontiguousarray(np.asarray(inputs["Q"], np.float32))
    cdd = np.ascontiguousarray(np.asarray(inputs["cdd"], np.float32))
    for name in ("bq", "bk", "bv", "lin_b", "lin1_b"):
        if not np.all(np.asarray(inputs[name]) == 0):
            raise NotImplementedError(f"nonzero bias {name} not supported")

    cbr, cb32 = _host_consts(
        inputs["Wq"], inputs["Wk"], inputs["Wv"], inputs["lin_w"], inputs["lin1_w"]
    )

    in_maps = []
    for i in range(NCORES):
        qs = np.zeros((NTILES * TILE, S * D), np.float32)
        qs[:BCH] = Q[i * BPC : (i + 1) * BPC].reshape(BCH, S * D)
        cs = np.zeros((NTILES * TILE, D), np.float32)
        cs[:BCH] = cdd[i * BPC : (i + 1) * BPC].reshape(BCH, D)
        in_maps.append(
            {
                "q": np.ascontiguousarray(qs),
                "cdd": np.ascontiguousarray(cs),
                "cbr": cbr,
                "cb32": cb32,
            }
        )

    res = run_bass_kernel_spmd(
        _get_nc(), in_maps, list(range(NCORES)), trace=trace, **kw
    )
    qa = np.stack([np.asarray(r["qa"])[:BCH] for r in res.results])
    ctx = np.stack([np.asarray(r["ctx"]) for r in res.results])
    q_a = qa.reshape(B, C, H, S, D)
    context = ctx.reshape(B, C, H, NT)
    return (context, q_a), res


def kernel(**inputs):
    (context, q_a), _ = run(inputs, trace=False)
    return context, q_a
